# revision 30
# baseline (speedup 1.0000x reference)
"""BAP classifier (attention-pooling + linear head) on 8 TRN2 NeuronCores.

Pipeline (reference math):
    A    = sigmoid(einsum('bchw,mc->bmhw', x, Wa) + ba)     # attention maps
    bap  = einsum('bchw,bmhw->bmc', x, A) / (H*W)           # attn-weighted pool
    out  = bap.reshape(B, M*C) @ Wc.T + bc                  # linear head

Sharding:
  Phase 1 — data-parallel over batch (8 batches/core).
  Phase 2 — Wc column-parallel (8192 columns per core); host sums the
    partial logits and applies scale + bias.

Precision: einsum1's x stream, the hw-tail (rows 128:196) of the einsum2
x^T operand, and phase-2's Wc are fp8 e3m4 (Wc pre-scaled by 256 to clear
the e3m4 subnormal floor; 1/256 folded into the host epilogue).  The rest
is bf16 with fp32 PSUM accumulation.  Measured rel err: ~1.79e-2.

DMA strategy (measured): the two HWDGE queues (sync / scalar) each sustain
~26 GB/s per SDMA engine and share the 16 engines round-robin; SWDGE
(gpsimd) is several times slower for partition-deficient transfers, so
phase 1 uses only the two HWDGE queues, each carrying a similar byte load
in consumption order: xn half -> x^T tiles -> feats store.
"""
import sys

if "/opt/trn_rl_repo" not in sys.path:
    sys.path.insert(0, "/opt/trn_rl_repo")

import numpy as np

import concourse.bacc as bacc
import concourse.mybir as mybir
from concourse.tile import TileContext
from concourse.bass_utils import run_bass_kernel_spmd
from concourse.masks import make_identity

B, C, H, W = 64, 2048, 14, 14
HW = H * W                     # 196
M, NCLS = 32, 396
NCORES = 8
BPC = B // NCORES              # 8 batches per core
CT = C // 128                  # 16 c-chunks
KTOT = M * C                   # 65536
KPC = KTOT // NCORES           # 8192 Wc columns per core
KT = KPC // 128                # 64 k-tiles per core in phase 2
NPAIR = BPC // 2               # 4 batch pairs per core

F32 = mybir.dt.float32
BF16 = mybir.dt.bfloat16
FP8 = mybir.dt.float8e3

WC_SCALE = 256.0               # Wc is sent as e3m4(Wc * 256)

# Run options (test harness may flip these; defaults are what grading uses).
TRACE = False
TRACE_INFO = {}
TRACE_RES = {}

_cache = {}


def _nc():
    return bacc.Bacc(
        "TRN2", target_bir_lowering=False, debug=False, num_devices=NCORES
    )


def _build_phase1():
    """Per-core: x shard -> raw feats (bf16, layout [(half b m), c]).

    Inputs (host-packed per core):
      xn  [2, 128, 2, CT, 2, HW] e3m4 — einsum1 stream, c permuted as
          c = p*CT + t; dims (half, p, pair-in-half, ct, b-in-pair, hw).
      xta [128, BPC, C]  bf16 — x^T rows hw 0:128, all batches.
      xtb [68, BPC, C]   e3m4 — x^T rows hw 128:196.
      wat [128, CT, M]   bf16, ba [M, 1] f32.

    Queue plan (arrival-ordered): sync carries the four per-pair xn loads
    then x^T half0 and the half0 store; scalar carries wat/ba then x^T
    half1 and its store.  Batches 4-7 are pooled FIRST because their x^T
    tiles (scalar queue) land before half0's (behind xn on sync).
    """
    nc = _nc()
    xn = nc.dram_tensor("xn", [2, 128, 2, CT, 2, HW], FP8, kind="ExternalInput")
    xta = nc.dram_tensor("xta", [128, BPC, C], BF16, kind="ExternalInput")
    xtb = nc.dram_tensor("xtb", [68, BPC, C], FP8, kind="ExternalInput")
    wat = nc.dram_tensor("wat", [128, CT, M], BF16, kind="ExternalInput")
    ba = nc.dram_tensor("ba", [M, 1], F32, kind="ExternalInput")
    feats = nc.dram_tensor("feats", [BPC * M, C], BF16, kind="ExternalOutput")

    with TileContext(nc) as tc:
        with (
            tc.tile_pool(name="const", bufs=1) as const,
            tc.tile_pool(name="xnpool", bufs=4) as xnpool,
            tc.tile_pool(name="xtapool", bufs=2) as xtapool,
            tc.tile_pool(name="xtbpool", bufs=2) as xtbpool,
            tc.tile_pool(name="apool", bufs=4) as apool,
            tc.tile_pool(name="atpool", bufs=8) as atpool,
            tc.tile_pool(name="fpool", bufs=2) as fpool,
            tc.tile_pool(name="ps_att", bufs=1, space="PSUM") as ps_att,
            tc.tile_pool(name="ps_tr", bufs=1, space="PSUM") as ps_tr,
            tc.tile_pool(name="ps_bap", bufs=5, space="PSUM") as ps_bap,
        ):
            ident = const.tile([M, M], BF16)
            make_identity(nc, ident)

            xn_sbs = []
            for pr in range(NPAIR):
                xn_sb = xnpool.tile([128, CT, 2, HW], FP8, tag="xn", name=f"xn{pr}")
                nc.sync.dma_start(out=xn_sb, in_=xn.ap()[pr // 2][:, pr % 2])
                xn_sbs.append(xn_sb)

            wat_sb = const.tile([128, CT, M], BF16)
            nc.scalar.dma_start(out=wat_sb, in_=wat.ap())
            ba_sb = const.tile([M, 1], F32)
            nc.scalar.dma_start(out=ba_sb, in_=ba.ap())

            # Warm the PE p-state while xn streams in: dummy transposes.
            warm_ps = ps_tr.tile([128, M], BF16, tag="ata", name="warm")
            for _ in range(20):
                nc.tensor.transpose(warm_ps[0:M, 0:M], ident, ident)

            # x^T tiles: half1 (batches 4-7) on scalar — lands first; half0
            # follows the xn stream on sync.
            xta_sbs = [None, None]
            xtb_sbs = [None, None]
            for h in (1, 0):
                eng = nc.scalar if h == 1 else nc.sync
                xta_sb = xtapool.tile([128, 4, C], BF16, tag="xta", name=f"xta{h}")
                eng.dma_start(out=xta_sb, in_=xta.ap()[:, 4 * h : 4 * h + 4, :])
                xta_sbs[h] = xta_sb
                xtb_sb = xtbpool.tile([68, 4, C], FP8, tag="xtb", name=f"xtb{h}")
                eng.dma_start(out=xtb_sb, in_=xtb.ap()[:, 4 * h : 4 * h + 4, :])
                xtb_sbs[h] = xtb_sb

            a_sbs = {}
            ats = {}
            featsqs = {}

            def einsum1(pr):
                att_ps = ps_att.tile([M, 2, HW], F32, tag="att", name=f"att{pr}")
                for ct in range(CT):
                    nc.tensor.matmul(
                        att_ps,
                        lhsT=wat_sb[:, ct, :],
                        rhs=xn_sbs[pr][:, ct, :, :],
                        start=(ct == 0),
                        stop=(ct == CT - 1),
                    )
                a_sb = apool.tile([M, 2, HW], BF16, tag="a_sb", name=f"a_sb{pr}")
                nc.scalar.activation(
                    out=a_sb,
                    in_=att_ps,
                    func=mybir.ActivationFunctionType.Sigmoid,
                    bias=ba_sb,
                )
                a_sbs[pr] = a_sb

            def transposes(pr):
                # A^T for both batches of pair pr (one pair of lookahead).
                a_sb = a_sbs[pr]
                pair_ats = []
                for b2 in range(2):
                    ata_ps = ps_tr.tile([128, M], BF16, tag="ata")
                    nc.tensor.transpose(
                        ata_ps, a_sb[:, b2, 0:128], ident[0:M, 0:M]
                    )
                    ata = atpool.tile([128, M], BF16, tag="ata_sb")
                    nc.scalar.copy(out=ata, in_=ata_ps)
                    atb_ps = ps_tr.tile([68, M], BF16, tag="atb")
                    nc.tensor.transpose(
                        atb_ps, a_sb[:, b2, 128:196], ident[0:M, 0:M]
                    )
                    atb = atpool.tile([68, M], BF16, tag="atb_sb")
                    nc.scalar.copy(out=atb, in_=atb_ps)
                    pair_ats.append((ata, atb))
                ats[pr] = pair_ats

            def emit_bap(pr):
                # feats staging per 4 batches: partition = 32*(b%4) + m
                half = pr // 2
                if half not in featsqs:
                    featsqs[half] = fpool.tile(
                        [128, C], BF16, tag="featsq", name=f"featsq{half}"
                    )
                featsq = featsqs[half]
                for b2 in range(2):
                    b = 2 * pr + b2
                    ata, atb = ats[pr][b2]
                    bap_ps = [
                        ps_bap.tile([M, 512], F32, tag="bap", name=f"bap_ps{nt}")
                        for nt in range(4)
                    ]
                    xta_sb = xta_sbs[b // 4]
                    xtb_sb = xtb_sbs[b // 4]
                    bl = b % 4
                    for nt in range(4):
                        nc.tensor.matmul(
                            bap_ps[nt],
                            lhsT=ata,
                            rhs=xta_sb[:, bl, 512 * nt : 512 * (nt + 1)],
                            start=True,
                            stop=False,
                        )
                    for nt in range(4):
                        nc.tensor.matmul(
                            bap_ps[nt],
                            lhsT=atb,
                            rhs=xtb_sb[:, bl, 512 * nt : 512 * (nt + 1)],
                            start=False,
                            stop=True,
                        )
                    row = 32 * (b % 4)
                    copy_eng = [
                        nc.vector.tensor_copy,
                        nc.scalar.copy,
                        nc.vector.tensor_copy,
                        nc.scalar.copy,
                    ]
                    for nt in range(4):
                        dst = featsq[row : row + 32, 512 * nt : 512 * (nt + 1)]
                        copy_eng[(b + nt) % 4](out=dst, in_=bap_ps[nt])

            def store_feats(half):
                eng = nc.sync if half == 0 else nc.scalar
                eng.dma_start(
                    out=feats.ap()[128 * half : 128 * (half + 1), :],
                    in_=featsqs[half],
                )

            einsum1(0)
            einsum1(1)
            einsum1(2)
            einsum1(3)
            transposes(2)
            transposes(3)
            emit_bap(2)
            transposes(0)
            emit_bap(3)
            store_feats(1)
            transposes(1)
            emit_bap(0)
            emit_bap(1)
            store_feats(0)
    nc.compile()
    return nc


def _build_phase2():
    """Per-core: featsT slice (bf16) x WcT slice (e3m4, pre-scaled x256)
    -> partial [B, NCLS] (fp32, carries the x256)."""
    nc = _nc()
    wrm = nc.dram_tensor("wrm", [128, 64], BF16, kind="ExternalInput")
    ft = nc.dram_tensor("ft", [128, KT, B], BF16, kind="ExternalInput")
    wct = nc.dram_tensor("wct", [128, KT, NCLS], FP8, kind="ExternalInput")
    part = nc.dram_tensor("part", [B, NCLS], F32, kind="ExternalOutput")

    # uneven wct chunks: a small first chunk lets the matmul chain start
    # right after ft chunk 0 lands instead of waiting for a full 8-kt load.
    BOUNDS = [0, 4, 12, 20, 28, 36, 44, 52, 64]
    NCH = len(BOUNDS) - 1
    OWNER = ["gpsimd", "scalar", "scalar", "gpsimd", "sync", "sync", "scalar", "scalar"]

    with TileContext(nc) as tc:
        with (
            tc.tile_pool(name="cpool", bufs=1) as cpool,
            tc.tile_pool(name="fpool", bufs=2) as fpool,
            tc.tile_pool(name="wpool", bufs=NCH) as wpool,
            tc.tile_pool(name="opool", bufs=1) as opool,
            tc.tile_pool(name="ps_out", bufs=1, space="PSUM") as ps_out,
        ):
            # tiny warm tensor rides first on sync; 16 dummy matmuls ramp the
            # PE p-state before the real chain.
            wrm_sb = cpool.tile([128, 64], BF16)
            nc.sync.dma_start(out=wrm_sb, in_=wrm.ap())
            warm_ps = ps_out.tile([B, B], F32, tag="warm")
            for _ in range(24):
                nc.tensor.matmul(
                    warm_ps, lhsT=wrm_sb, rhs=wrm_sb, start=True, stop=True
                )

            ft_sbs = []
            for h in range(2):
                ft_sb = fpool.tile([128, KT // 2, B], BF16, tag="ft", name=f"ft{h}")
                nc.sync.dma_start(
                    out=ft_sb, in_=ft.ap()[:, h * (KT // 2) : (h + 1) * (KT // 2), :]
                )
                ft_sbs.append(ft_sb)

            engs = {"sync": nc.sync, "scalar": nc.scalar, "gpsimd": nc.gpsimd}
            w_sbs = []
            for kc in range(NCH):
                k0, k1 = BOUNDS[kc], BOUNDS[kc + 1]
                w_sb = wpool.tile(
                    [128, 12, NCLS], FP8, tag="w", name=f"w_sb{kc}"
                )
                engs[OWNER[kc]].dma_start(
                    out=w_sb[:, 0 : k1 - k0, :],
                    in_=wct.ap()[:, k0:k1, :],
                )
                w_sbs.append(w_sb)

            out_ps = ps_out.tile([B, NCLS], F32, tag="out")
            for kc in range(NCH):
                for kl in range(BOUNDS[kc + 1] - BOUNDS[kc]):
                    kt = BOUNDS[kc] + kl
                    nc.tensor.matmul(
                        out_ps,
                        lhsT=ft_sbs[kt // (KT // 2)][:, kt % (KT // 2), :],
                        rhs=w_sbs[kc][:, kl, :],
                        start=(kt == 0),
                        stop=(kt == KT - 1),
                    )
            out_sb = opool.tile([B, NCLS], F32)
            nc.scalar.copy(out=out_sb, in_=out_ps)
            nc.sync.dma_start(out=part.ap(), in_=out_sb)
    nc.compile()
    return nc


def _install_ntff_hook():
    import types

    import trn_agent_boot.trn_boot as tb
    import concourse.bass_utils as bu

    hook = tb._ntff_profile_via_ctypes("/opt/axon/libaxon_pjrt.so")
    mod = types.ModuleType("antenv.axon_hooks")
    mod.get_axon_ntff_profile_hook = lambda: hook
    sys.modules["antenv.axon_hooks"] = mod
    bu.upload_artifacts = lambda tmpdir: "(skipped)"


def _run(nc, in_maps, label):
    core_ids = list(range(NCORES))
    if TRACE:
        _install_ntff_hook()
        res = run_bass_kernel_spmd(nc, in_maps, core_ids, trace=True)
        TRACE_INFO[label] = res.exec_time_ns
        TRACE_RES[label] = res
    else:
        res = run_bass_kernel_spmd(nc, in_maps, core_ids)
    return res.results


def kernel(x, Wa, ba, Wc, bc):
    import ml_dtypes

    bf16 = np.dtype(ml_dtypes.bfloat16)
    e3m4 = np.dtype(ml_dtypes.float8_e3m4)

    x3 = np.ascontiguousarray(x, dtype=np.float32).reshape(B, C, HW)
    xbf = x3.astype(bf16)
    x8 = x3.astype(e3m4)
    # wat[p, t, m] = Wa[m, p*CT + t] — matches the kernel's permuted c layout
    wat = np.ascontiguousarray(Wa.T, dtype=np.float32).astype(bf16).reshape(
        128, CT, M
    )
    ba2 = np.ascontiguousarray(ba, dtype=np.float32).reshape(M, 1)
    wct = (
        np.ascontiguousarray(Wc.T, dtype=np.float32) * WC_SCALE
    ).astype(e3m4)  # [KTOT, NCLS]
    wrm = np.zeros((128, 64), dtype=bf16)

    if "p1" not in _cache:
        _cache["p1"] = _build_phase1()
    if "p2" not in _cache:
        _cache["p2"] = _build_phase2()

    in1 = []
    for i in range(NCORES):
        sl = slice(i * BPC, (i + 1) * BPC)
        # xn[hf, p, pr2, t, b2, hw] = x8[4hf + 2*pr2 + b2, p*CT+t, hw]
        xn = np.ascontiguousarray(
            x8[sl].reshape(2, 2, 2, 128, CT, HW).transpose(0, 3, 1, 4, 2, 5)
        )
        xt8 = x8[sl].transpose(2, 0, 1)  # [HW, BPC, C] fp8
        xtb_ = xbf[sl].transpose(2, 0, 1)  # reuse bf16 for hw 0:128
        in1.append(
            {
                "xn": xn,
                "xta": np.ascontiguousarray(xtb_[0:128]),
                "xtb": np.ascontiguousarray(xt8[128:196]),
                "wat": wat,
                "ba": ba2,
            }
        )
    res1 = _run(_cache["p1"], in1, "phase1")
    # feats rows: half h holds batches 4h..4h+3 as (b%4)*M + m
    feats = np.concatenate(
        [r["feats"].reshape(2, 4, M, C) for r in res1], axis=0
    ).reshape(B, M, C).reshape(B, KTOT)

    # ft[p, t, b] = feats[b, kslice + t*128 + p] (partition-major, bf16)
    featsT = np.ascontiguousarray(feats.T)  # [KTOT, B]
    in2 = [
        {
            "wrm": wrm,
            "ft": np.ascontiguousarray(
                featsT[i * KPC : (i + 1) * KPC].reshape(KT, 128, B).transpose(
                    1, 0, 2
                )
            ),
            "wct": np.ascontiguousarray(
                wct[i * KPC : (i + 1) * KPC].reshape(KT, 128, NCLS).transpose(
                    1, 0, 2
                )
            ),
        }
        for i in range(NCORES)
    ]
    res2 = _run(_cache["p2"], in2, "phase2")
    parts = np.stack([r["part"] for r in res2], axis=0)  # [NCORES, B, NCLS]

    logits = parts.sum(axis=0) / float(HW * WC_SCALE) + np.asarray(
        bc, dtype=np.float32
    )
    return logits.astype(np.float32)
